# revision 1
# baseline (speedup 1.0000x reference)
"""Trainium2 Bass kernel for nn_AdaptiveAutoCorrelation (8-core data-parallel).

Per-core (one batch element b):
  1. LayerNorm(q), LayerNorm(k) over E=64 segments ([128, 12, 512] mega),
     stats f32 on DVE, megas written fp16.
  2. Avg-pool to scales 2,4 on DVE (strided add + 0.5 scale), fp16.
  3. rFFT of q,k per scale as fp16 matmuls against host-built DFT chain
     matrices (one DMA per (scale, f-tile, re/im) chain); PSUM drained to
     SBUF f32 by ACT.
  4. Spectral whitening qf*conj(kf)*rsqrt(|kf|^2) + (h,e)-reduction via
     stt accum -> S[f] stored fp16.
  5. mean_corr = S @ M (irfft+interp+scale-weights+mean folded into M,
     M prescaled by 2^14 host-side to stay fp16-normal; undone in the
     mc_row copy). mc matmuls emitted one pair behind the DFT stream.
  6. top-7 via DVE max/max_index, softmax; 7x3 indirect gathers from a
     host-built sliding-window fp16 buffer + weighted MAC on DVE.

The (H,E)-mean/clip swap is exact for this model: |corr| <= ~3.7 << 10.
fp16 error budget on mean_corr ~2.4e-5 << min rank-7/8 gap 2.6e-4.
"""
import math

import numpy as np

L = 1536
H, E = 8, 64
R = H * E  # 512
B = 8
NT = L // 128  # 12 l-tiles
SCALES = [1, 2, 4]
KT = [12, 6, 3]  # contraction tiles per scale (pooled-first)
FBINS = [L // s // 2 + 1 for s in SCALES]  # [769, 385, 193]
FT = [(f + 127) // 128 for f in FBINS]  # f-tiles per re/im block: [7, 4, 2]
NFT = 2 * sum(FT)  # 26 total f-tiles
TOPK = int(math.log(L))  # 7
LN_EPS = 1e-5
GPK = 6  # tiles packed per gather row (6KB rows)
NGRP = NT // GPK  # 3 gathers per delay
NW = 2 * L - 128 * (GPK - 1)  # 2688 rows in the sliding-window gather buffer
MC_SHIFT = 2.0 ** -14  # undo host-side M prescale (exact power of two)

# global ftile index bases (for S / M layout): per scale, re tiles then im
_FT_BASE = []
_acc = 0
for _s in range(len(SCALES)):
    _FT_BASE.append((_acc, _acc + FT[_s]))
    _acc += 2 * FT[_s]

_CACHE = {}


def _build_constants(scale_weights, frequency_filter):
    """D chains per scale [2*nf,128,nkt*128] fp16, M [NFT,128,L] fp16."""
    f_sig = 1.0 / (1.0 + np.exp(-np.float64(frequency_filter[0])))
    sw = np.asarray(scale_weights[: len(SCALES)], np.float64)
    w = np.exp(sw - sw.max())
    w = w / w.sum()

    d_chains = []
    M = np.zeros((NFT * 128, L), np.float64)
    for si, s in enumerate(SCALES):
        Ls = L // s
        F = FBINS[si]
        nf = FT[si]
        nkt = KT[si]
        t = np.arange(Ls)[:, None]
        f = np.arange(F)[None, :]
        ang = 2.0 * np.pi * t * f / Ls
        Dre = np.zeros((Ls, nf * 128))
        Dim = np.zeros((Ls, nf * 128))
        Dre[:, :F] = np.cos(ang)
        Dim[:, :F] = -np.sin(ang)
        # chain layout: [lf, p, kt*128 + fc] = blk[kt*128 + p, j*128 + fc]
        cr = Dre.reshape(nkt, 128, nf, 128).transpose(2, 1, 0, 3)
        ci = Dim.reshape(nkt, 128, nf, 128).transpose(2, 1, 0, 3)
        ch = np.concatenate([cr, ci], axis=0).reshape(2 * nf, 128, nkt * 128)
        d_chains.append(np.ascontiguousarray(ch.astype(np.float16)))

        reb, imb = _FT_BASE[si]
        tt = np.arange(Ls)[None, :]
        cf = np.where((f.T == 0) | (f.T == F - 1), 1.0, 2.0)
        ang2 = 2.0 * np.pi * f.T * tt / Ls
        Mre = cf * np.cos(ang2) / Ls  # [F, Ls]
        Mim = -cf * np.sin(ang2) / Ls
        if Ls != L:
            P = np.zeros((Ls, L))
            co = np.clip((np.arange(L) + 0.5) * (Ls / L) - 0.5, 0, Ls - 1)
            lo = np.floor(co).astype(int)
            hi = np.minimum(lo + 1, Ls - 1)
            fr = co - lo
            P[lo, np.arange(L)] += 1 - fr
            P[hi, np.arange(L)] += fr
            Mre = Mre @ P
            Mim = Mim @ P
        scale = w[si] * f_sig / R * 16384.0  # 2^14 prescale for fp16 range
        M[reb * 128 : reb * 128 + F] = Mre * scale
        M[imb * 128 : imb * 128 + F] = Mim * scale

    M_t = np.ascontiguousarray(M.reshape(NFT, 128, L).astype(np.float16))
    # pool-by-2 packing matrices: P2a -> out cols [0,64), P2b -> [64,128)
    P2 = np.zeros((2, 128, 128), np.float16)
    for t_ in range(128):
        P2[0, t_, t_ // 2] = 0.5
        P2[1, t_, 64 + t_ // 2] = 0.5
    return d_chains, M_t, P2


def _build_graph():
    import concourse.bacc as bacc
    import concourse.bass as bass
    import concourse.mybir as mybir
    import concourse.tile as tile

    AF = mybir.ActivationFunctionType
    OP = mybir.AluOpType
    f32 = mybir.dt.float32
    f16 = mybir.dt.float16
    u32 = mybir.dt.uint32

    nc = bacc.Bacc("TRN2", debug=False)
    q_d = nc.dram_tensor("q", [NT, 128, R], f32, kind="ExternalInput")
    k_d = nc.dram_tensor("k", [NT, 128, R], f32, kind="ExternalInput")
    vw_d = nc.dram_tensor("vw", [NW, GPK * R], f16, kind="ExternalInput")
    d_ds = [
        nc.dram_tensor(
            f"dmat{si}", [2 * FT[si], 128, KT[si] * 128], f16,
            kind="ExternalInput",
        )
        for si in range(len(SCALES))
    ]
    m_d = nc.dram_tensor("mmat", [NFT, 128, L], f16, kind="ExternalInput")
    p_d = nc.dram_tensor("pmat", [2, 128, 128], f16, kind="ExternalInput")
    o_d = nc.dram_tensor("out", [NT, 128, R], f32, kind="ExternalOutput")

    with tile.TileContext(nc) as tc:
        with (
            tc.tile_pool(name="qk", bufs=1) as qkpool,
            tc.tile_pool(name="small", bufs=1) as spool,
        ):
            eps_ln = spool.tile([128, 1], f32, tag="eps_ln")
            nc.vector.memset(eps_ln[:], LN_EPS)
            eps_mag = spool.tile([128, 1], f32, tag="eps_mag")
            nc.vector.memset(eps_mag[:], 1e-30)
            p2a = spool.tile([128, 128], f16, tag="p2a")
            p2b = spool.tile([128, 128], f16, tag="p2b")
            nc.sync.dma_start(p2a[:], p_d.ap()[0])
            nc.sync.dma_start(p2b[:], p_d.ap()[1])

            # ---- load + layernorm (mega per tensor, fp16 out) ----
            xn = {}
            with tc.tile_pool(name="lnwork", bufs=2) as wpool:
                for name, src in (("q", q_d), ("k", k_d)):
                    raw = wpool.tile([128, NT, R], f32, tag="raw")
                    nc.sync.dma_start(
                        raw[:], src.ap().rearrange("t p r -> p t r")
                    )
                    x4 = raw[:].rearrange("p t (h e) -> p t h e", e=E)
                    sq = wpool.tile([128, NT, R], f32, tag="sq")
                    nc.scalar.activation(sq[:], raw[:], AF.Square)
                    stat = wpool.tile([128, 96], f32, tag="stat")
                    nc.vector.tensor_reduce(
                        stat[:], x4, mybir.AxisListType.X, OP.add
                    )
                    msq = wpool.tile([128, 96], f32, tag="msq")
                    nc.vector.tensor_reduce(
                        msq[:], sq[:].rearrange("p t (h e) -> p t h e", e=E),
                        mybir.AxisListType.X, OP.add,
                    )
                    mean = wpool.tile([128, 96], f32, tag="mean")
                    nc.vector.tensor_scalar_mul(mean[:], stat[:], 1.0 / E)
                    m2 = wpool.tile([128, 96], f32, tag="m2")
                    nc.vector.tensor_mul(m2[:], mean[:], mean[:])
                    var = wpool.tile([128, 96], f32, tag="var")
                    nc.vector.scalar_tensor_tensor(
                        var[:], msq[:], 1.0 / E, m2[:],
                        op0=OP.mult, op1=OP.subtract,
                    )
                    std = wpool.tile([128, 96], f32, tag="std")
                    nc.scalar.activation(std[:], var[:], AF.Sqrt, bias=eps_ln[:])
                    rstd = wpool.tile([128, 96], f32, tag="rstd")
                    nc.vector.reciprocal(rstd[:], std[:])
                    mega = qkpool.tile(
                        [128, NT, R], f16, tag=f"{name}mega", name=f"{name}mega"
                    )
                    mg4 = mega[:].rearrange("p t (h e) -> p t h e", e=E)
                    mean4 = mean[:].rearrange("p (t h o) -> p t h o", t=NT, o=1)
                    rstd4 = rstd[:].rearrange("p (t h o) -> p t h o", t=NT, o=1)
                    x4b, mean_b = bass.broadcast_tensor_aps(x4, mean4)
                    nc.vector.tensor_tensor(mg4, x4b, mean_b, OP.subtract)
                    _, rstd_b = bass.broadcast_tensor_aps(mg4, rstd4)
                    nc.vector.tensor_tensor(mg4, mg4, rstd_b, OP.mult)
                    xn[(name, 0)] = mega

            # pooled megas (filled on DVE, emission deferred into pair loop)
            for name in ("q", "k"):
                for si, nkt in ((1, 6), (2, 3)):
                    xn[(name, si)] = qkpool.tile(
                        [128, nkt, R], f16, tag=f"{name}p{si}",
                        name=f"{name}p{si}",
                    )

            # ---- DFT + spectral + pipelined irfft (mean_corr) ----
            S16 = spool.tile([128, 32], f16, tag="s16")
            nc.vector.memset(S16[:], 0.0)
            with (
                tc.tile_pool(name="psum", bufs=5, space="PSUM") as pp,
                tc.tile_pool(name="mcpsum", bufs=1, space="PSUM") as mcp,
                tc.tile_pool(name="dstream", bufs=3) as dpool,
                tc.tile_pool(name="mstream", bufs=2) as mpool,
                tc.tile_pool(name="spec", bufs=2) as scp,
            ):
                mc_ps = [
                    mcp.tile([1, 512], f32, tag=f"mc{nt}", name=f"mc{nt}")
                    for nt in range(3)
                ]

                def emit_pools():
                    # avg-pool by 2 crosses partitions: PE packing matmuls
                    for name in ("q", "k"):
                        for si, nkt in ((1, 6), (2, 3)):
                            srcm = xn[(name, si - 1)]
                            dst = xn[(name, si)]
                            for j2 in range(nkt):
                                ps = pp.tile(
                                    [128, 512], f32, tag="dftps", name="poolps"
                                )
                                nc.tensor.matmul(
                                    ps[:], p2a[:], srcm[:, 2 * j2, :],
                                    start=True, stop=False,
                                )
                                nc.tensor.matmul(
                                    ps[:], p2b[:], srcm[:, 2 * j2 + 1, :],
                                    start=False, stop=True,
                                )
                                nc.scalar.activation(
                                    dst[:, j2, :], ps[:], AF.Copy
                                )
                pair_list = []
                for si in range(len(SCALES)):
                    reb, imb = _FT_BASE[si]
                    for j in range(FT[si]):
                        pair_list.append((si, j, reb + j, imb + j))
                n_pairs = len(pair_list)

                def is_orphan(si2, j2):
                    # last f-tile of scales 1,2 holds a single (Nyquist) bin
                    # whose imaginary part is exactly zero
                    return si2 < 2 and j2 == FT[si2] - 1

                def emit_mc(pi2, first_mm):
                    si2, j2, ftr2, fti2 = pair_list[pi2]
                    fts = (ftr2,) if is_orphan(si2, j2) else (ftr2, fti2)
                    for ft in fts:
                        mtile = mpool.tile([128, L], f16, tag="mtile")
                        nc.sync.dma_start(mtile[:], m_d.ap()[ft])
                        for nt in range(3):
                            nc.tensor.matmul(
                                mc_ps[nt][:], S16[:, ft : ft + 1],
                                mtile[:, nt * 512 : (nt + 1) * 512],
                                start=first_mm,
                                stop=(
                                    pi2 == n_pairs - 1 and ft == fti2
                                    and nt == 2
                                ),
                                skip_group_check=True,
                            )
                        first_mm = False
                    return first_mm

                first_mm = True
                for pi, (si, j, ftr, fti) in enumerate(pair_list):
                    nkt = KT[si]
                    qx = xn[("q", si)]
                    kx = xn[("k", si)]
                    orphan = is_orphan(si, j)
                    psl = {}
                    # q chains first so pair-0 PE work overlaps LN(k)
                    chains = (
                        (("qre", qx, j), ("kre", kx, j))
                        if orphan
                        else (
                            ("qre", qx, j), ("qim", qx, FT[si] + j),
                            ("kre", kx, j), ("kim", kx, FT[si] + j),
                        )
                    )
                    for nm, xm, lf in chains:
                        dch = dpool.tile([128, nkt, 128], f16, tag=f"d{si}")
                        nc.sync.dma_start(
                            dch[:].rearrange("p a b -> p (a b)"),
                            d_ds[si].ap()[lf],
                        )
                        ps = pp.tile([128, 512], f32, tag="dftps", name=f"ps{nm}")
                        for kt in range(nkt):
                            nc.tensor.matmul(
                                ps[:], dch[:, kt, :], xm[:, kt, :],
                                start=(kt == 0), stop=(kt == nkt - 1),
                            )
                        sb = scp.tile([128, 512], f32, tag=f"{nm}S")
                        nc.scalar.activation(sb[:], ps[:], AF.Copy)
                        psl[nm] = sb
                    # mc matmuls for the previous pair (PE stays dense)
                    if pi > 0:
                        first_mm = emit_mc(pi - 1, first_mm)
                    if pi == 0:
                        emit_pools()
                    qreS, kreS = psl["qre"], psl["kre"]
                    sq1 = scp.tile([128, 512], f32, tag="sq1")
                    nc.scalar.activation(sq1[:], kreS[:], AF.Square)
                    if orphan:
                        # im == 0: S_re = sum qre*kre/|kre|, S_im = 0 (memset)
                        mag = scp.tile([128, 512], f32, tag="mag")
                        nc.scalar.activation(
                            mag[:], sq1[:], AF.Sqrt, bias=eps_mag[:, 0:1]
                        )
                        rs = scp.tile([128, 512], f32, tag="rs")
                        nc.vector.reciprocal_approx_fast(rs[:], mag[:])
                        khr = scp.tile([128, 512], f32, tag="khr")
                        nc.vector.tensor_mul(khr[:], kreS[:], rs[:])
                        scr = scp.tile([128, 512], f32, tag="scr")
                        a1 = scp.tile([128, 1], f32, tag="a1")
                        nc.vector.scalar_tensor_tensor(
                            scr[:], qreS[:], 0.0, khr[:], op0=OP.bypass,
                            op1=OP.mult, accum_out=a1[:],
                        )
                        nc.vector.tensor_copy(S16[:, ftr : ftr + 1], a1[:])
                        continue
                    qimS, kimS = psl["qim"], psl["kim"]
                    sq2 = scp.tile([128, 512], f32, tag="sq2")
                    nc.scalar.activation(sq2[:], kimS[:], AF.Square)
                    mag2 = scp.tile([128, 512], f32, tag="mag2")
                    nc.vector.tensor_add(mag2[:], sq1[:], sq2[:])
                    mag = scp.tile([128, 512], f32, tag="mag")
                    nc.scalar.activation(
                        mag[:], mag2[:], AF.Sqrt, bias=eps_mag[:, 0:1]
                    )
                    rs = scp.tile([128, 512], f32, tag="rs")
                    nc.vector.reciprocal_approx_fast(rs[:], mag[:])
                    khr = scp.tile([128, 512], f32, tag="khr")
                    khi = scp.tile([128, 512], f32, tag="khi")
                    nc.vector.tensor_mul(khr[:], kreS[:], rs[:])
                    nc.vector.tensor_mul(khi[:], kimS[:], rs[:])
                    scr = scp.tile([128, 512], f32, tag="scr")
                    scr2 = scp.tile([128, 512], f32, tag="scr2")
                    a1 = scp.tile([128, 1], f32, tag="a1")
                    a2 = scp.tile([128, 1], f32, tag="a2")
                    a3 = scp.tile([128, 1], f32, tag="a3")
                    a4 = scp.tile([128, 1], f32, tag="a4")
                    nc.vector.scalar_tensor_tensor(
                        scr[:], qreS[:], 0.0, khr[:], op0=OP.bypass,
                        op1=OP.mult, accum_out=a1[:],
                    )
                    nc.vector.scalar_tensor_tensor(
                        scr2[:], qimS[:], 0.0, khi[:], op0=OP.bypass,
                        op1=OP.mult, accum_out=a2[:],
                    )
                    nc.vector.tensor_add(S16[:, ftr : ftr + 1], a1[:], a2[:])
                    nc.vector.scalar_tensor_tensor(
                        scr[:], qimS[:], 0.0, khr[:], op0=OP.bypass,
                        op1=OP.mult, accum_out=a3[:],
                    )
                    nc.vector.scalar_tensor_tensor(
                        scr2[:], qreS[:], 0.0, khi[:], op0=OP.bypass,
                        op1=OP.mult, accum_out=a4[:],
                    )
                    nc.vector.tensor_sub(S16[:, fti : fti + 1], a3[:], a4[:])
                first_mm = emit_mc(n_pairs - 1, first_mm)

                mc_row = spool.tile([1, L], f32, tag="mcrow")
                for nt in range(3):
                    nc.vector.tensor_scalar_mul(
                        mc_row[:, nt * 512 : (nt + 1) * 512], mc_ps[nt][:],
                        MC_SHIFT,
                    )

            # ---- top-7 + softmax ----
            mc8 = spool.tile([1, 8], f32, tag="mc8")
            mcidx = spool.tile([1, 8], u32, tag="mcidx")
            nc.vector.max(mc8[:], mc_row[:])
            nc.vector.max_index(mcidx[:], mc8[:], mc_row[:])
            mc8c = spool.tile([1, 8], f32, tag="mc8c")
            nc.vector.tensor_copy(mc8c[:], mc8[:])
            mcidxc = spool.tile([1, 8], u32, tag="mcidxc")
            nc.vector.tensor_copy(mcidxc[:], mcidx[:])
            negmax = spool.tile([1, 1], f32, tag="negmax")
            nc.vector.tensor_scalar_mul(negmax[:], mc8c[:, 0:1], -1.0)
            e7 = spool.tile([1, TOPK], f32, tag="e7")
            nc.scalar.activation(e7[:], mc8c[:, 0:TOPK], AF.Exp, bias=negmax[:])
            ssum = spool.tile([1, 1], f32, tag="ssum")
            nc.vector.tensor_reduce(ssum[:], e7[:], mybir.AxisListType.X, OP.add)
            rsum = spool.tile([1, 1], f32, tag="rsum")
            nc.vector.reciprocal(rsum[:], ssum[:])
            nw = spool.tile([1, TOPK], f32, tag="nw")
            nc.vector.tensor_scalar_mul(nw[:], e7[:], rsum[:, 0:1])
            nw128 = spool.tile([128, TOPK], f32, tag="nw128")
            nc.gpsimd.partition_broadcast(nw128[:], nw[:])
            d128a = spool.tile([128, TOPK], u32, tag="d128a")
            nc.gpsimd.partition_broadcast(d128a[:], mcidxc[:, 0:TOPK])
            iotas = []
            for g in range(NGRP):
                it = spool.tile([128, 1], u32, tag=f"iota{g}", name=f"iota{g}")
                nc.gpsimd.iota(
                    it[:], pattern=[[0, 1]], base=128 * GPK * g,
                    channel_multiplier=1,
                )
                iotas.append(it)

            # ---- gather (3 packed indirect gathers per delay) + MAC ----
            with tc.tile_pool(name="gather", bufs=4) as gpool:
                acc = gpool.tile([128, NT, R], f32, tag="acc", bufs=1)
                for g in range(NGRP):
                    for kk in range(TOPK):
                        idx = gpool.tile([128, 1], u32, tag="idx")
                        nc.vector.tensor_tensor(
                            idx[:], iotas[g][:], d128a[:, kk : kk + 1], OP.add
                        )
                        slot = gpool.tile(
                            [128, GPK * R], f16, tag="slot", bufs=4
                        )
                        nc.gpsimd.indirect_dma_start(
                            out=slot[:],
                            out_offset=None,
                            in_=vw_d.ap(),
                            in_offset=bass.IndirectOffsetOnAxis(
                                ap=idx[:, 0:1], axis=0
                            ),
                        )
                        av = acc[:].rearrange("p t r -> p (t r)")[
                            :, GPK * R * g : GPK * R * (g + 1)
                        ]
                        if kk == 0:
                            nc.vector.tensor_scalar_mul(
                                av, slot[:], nw128[:, 0:1]
                            )
                        else:
                            nc.vector.scalar_tensor_tensor(
                                av, slot[:], nw128[:, kk : kk + 1], av,
                                op0=OP.mult, op1=OP.add,
                            )
                    # stream this group's output while the next group gathers
                    for c in range(GPK):
                        kt = GPK * g + c
                        nc.sync.dma_start(o_d.ap()[kt], acc[:, kt, :])

    nc.compile()
    return nc


def _get_graph():
    if "nc" not in _CACHE:
        _CACHE["nc"] = _build_graph()
    return _CACHE["nc"]


def _make_in_maps(queries, keys, values, scale_weights, frequency_filter):
    d_chains, M_t, P2 = _build_constants(
        np.asarray(scale_weights, np.float64),
        np.asarray(frequency_filter, np.float64),
    )
    q = np.ascontiguousarray(np.asarray(queries, np.float32).reshape(B, NT, 128, R))
    k = np.ascontiguousarray(np.asarray(keys, np.float32).reshape(B, NT, 128, R))
    v = np.asarray(values, np.float32).reshape(B, L, R)
    vv = np.concatenate([v, v], axis=1).astype(np.float16)  # [B, 2L, R]
    # sliding-window buffer: vw[b, i, c, :] = vv[b, i + 128*c, :], c < GPK
    st = vv.strides
    vw = np.lib.stride_tricks.as_strided(
        vv, shape=(B, NW, GPK, R), strides=(st[0], st[1], 128 * st[1], st[2])
    )
    in_maps = []
    for b in range(B):
        m = {
            "q": q[b],
            "k": k[b],
            "vw": np.ascontiguousarray(vw[b]).reshape(NW, GPK * R),
            "mmat": M_t,
        }
        m["pmat"] = P2
        for si in range(len(SCALES)):
            m[f"dmat{si}"] = d_chains[si]
        in_maps.append(m)
    return in_maps


def kernel(queries, keys, values, scale_weights, frequency_filter, attn_mask=None):
    from concourse.bass_utils import run_bass_kernel_spmd

    nc = _get_graph()
    in_maps = _make_in_maps(queries, keys, values, scale_weights, frequency_filter)
    res = run_bass_kernel_spmd(nc, in_maps, core_ids=list(range(B)))
    out = np.stack(
        [np.asarray(res.results[b]["out"]).reshape(L, H, E) for b in range(B)]
    )
    return out.astype(np.float32)



# revision 4
# speedup vs baseline: 1.0574x; 1.0574x over previous
"""Trainium2 Bass kernel for nn_AdaptiveAutoCorrelation (8-core data-parallel).

v2 — restructured from the 293us baseline:
  * LayerNorm is pipelined per l-tile (q stats/normalize on DVE, k on
    GpSimd, squares/sqrt on ACT) so the first DFT matmul issues ~3us in
    instead of 81us.
  * D-chain tiles are loaded once per (scale, f-tile, re/im) and shared
    by the q and k chains (halves D DMA).
  * Spectral whitening runs in fp16 on DVE (f32 accum), psum drained by
    ACT (drain doubles as f32->fp16 convert).
  * mean_corr matmuls lag the DFT stream by 2 pairs (no PE queue
    head-of-line blocking on the spectral chain).
  * The 7-delay weighted aggregation runs on the PE (nw_k * I stationary
    against gathered fp16 slots, accumulated in PSUM) instead of DVE.
"""
import math

import numpy as np

L = 1536
H, E = 8, 64
R = H * E  # 512
B = 8
NT = L // 128  # 12 l-tiles
SCALES = [1, 2, 4]
KT = [12, 6, 3]  # contraction tiles per scale (pooled-first)
FBINS = [L // s // 2 + 1 for s in SCALES]  # [769, 385, 193]
FT = [(f + 127) // 128 for f in FBINS]  # f-tiles per re/im block: [7, 4, 2]
NFT = 2 * sum(FT)  # 26 total f-tiles
TOPK = int(math.log(L))  # 7
LN_EPS = 1e-5
GPK = 6  # tiles packed per gather row (6KB rows)
NGRP = NT // GPK  # 2 gathers per delay
NW = 2 * L - 128 * (GPK - 1)  # 2688 rows in the sliding-window gather buffer
MC_SHIFT = 2.0 ** -14  # undo host-side M prescale (exact power of two)
MAG_EPS2 = 1e-6  # |kf|^2 floor: keeps rs finite for zero-padded bins

# global ftile index bases (for S / M layout): per scale, re tiles then im
_FT_BASE = []
_acc = 0
for _s in range(len(SCALES)):
    _FT_BASE.append((_acc, _acc + FT[_s]))
    _acc += 2 * FT[_s]

_CACHE = {}


def _build_constants(scale_weights, frequency_filter):
    """D chains per scale [2*nf,128,nkt*128] fp16, M [NFT,128,L] fp16."""
    f_sig = 1.0 / (1.0 + np.exp(-np.float64(frequency_filter[0])))
    sw = np.asarray(scale_weights[: len(SCALES)], np.float64)
    w = np.exp(sw - sw.max())
    w = w / w.sum()

    d_chains = []
    M = np.zeros((NFT * 128, L), np.float64)
    for si, s in enumerate(SCALES):
        Ls = L // s
        F = FBINS[si]
        nf = FT[si]
        nkt = KT[si]
        t = np.arange(Ls)[:, None]
        f = np.arange(F)[None, :]
        ang = 2.0 * np.pi * t * f / Ls
        Dre = np.zeros((Ls, nf * 128))
        Dim = np.zeros((Ls, nf * 128))
        Dre[:, :F] = np.cos(ang)
        Dim[:, :F] = -np.sin(ang)
        # chain layout: [lf, p, kt*128 + fc] = blk[kt*128 + p, j*128 + fc]
        cr = Dre.reshape(nkt, 128, nf, 128).transpose(2, 1, 0, 3)
        ci = Dim.reshape(nkt, 128, nf, 128).transpose(2, 1, 0, 3)
        ch = np.concatenate([cr, ci], axis=0).reshape(2 * nf, 128, nkt * 128)
        d_chains.append(np.ascontiguousarray(ch.astype(np.float16)))

        reb, imb = _FT_BASE[si]
        tt = np.arange(Ls)[None, :]
        cf = np.where((f.T == 0) | (f.T == F - 1), 1.0, 2.0)
        ang2 = 2.0 * np.pi * f.T * tt / Ls
        Mre = cf * np.cos(ang2) / Ls  # [F, Ls]
        Mim = -cf * np.sin(ang2) / Ls
        if Ls != L:
            P = np.zeros((Ls, L))
            co = np.clip((np.arange(L) + 0.5) * (Ls / L) - 0.5, 0, Ls - 1)
            lo = np.floor(co).astype(int)
            hi = np.minimum(lo + 1, Ls - 1)
            fr = co - lo
            P[lo, np.arange(L)] += 1 - fr
            P[hi, np.arange(L)] += fr
            Mre = Mre @ P
            Mim = Mim @ P
        scale = w[si] * f_sig / R * 16384.0  # 2^14 prescale for fp16 range
        M[reb * 128 : reb * 128 + F] = Mre * scale
        M[imb * 128 : imb * 128 + F] = Mim * scale

    M_t = np.ascontiguousarray(M.reshape(NFT, 128, L).astype(np.float16))
    # pool-by-2 packing matrices: P2a -> out cols [0,64), P2b -> [64,128)
    P2 = np.zeros((2, 128, 128), np.float16)
    for t_ in range(128):
        P2[0, t_, t_ // 2] = 0.5
        P2[1, t_, 64 + t_ // 2] = 0.5
    I128 = np.eye(128, dtype=np.float16)
    return d_chains, M_t, P2, I128


def _build_graph():
    import concourse.bacc as bacc
    import concourse.bass as bass
    import concourse.mybir as mybir
    import concourse.tile as tile

    AF = mybir.ActivationFunctionType
    OP = mybir.AluOpType
    f32 = mybir.dt.float32
    f16 = mybir.dt.float16
    u32 = mybir.dt.uint32

    nc = bacc.Bacc("TRN2", debug=False)
    q_d = nc.dram_tensor("q", [NT, 128, R], f32, kind="ExternalInput")
    k_d = nc.dram_tensor("k", [NT, 128, R], f32, kind="ExternalInput")
    vw_d = nc.dram_tensor("vw", [NW, GPK * R], f16, kind="ExternalInput")
    d_ds = [
        nc.dram_tensor(
            f"dmat{si}", [2 * FT[si], 128, KT[si] * 128], f16,
            kind="ExternalInput",
        )
        for si in range(len(SCALES))
    ]
    m_d = nc.dram_tensor("mmat", [NFT, 128, L], f16, kind="ExternalInput")
    p_d = nc.dram_tensor("pmat", [2, 128, 128], f16, kind="ExternalInput")
    i_d = nc.dram_tensor("imat", [128, 128], f16, kind="ExternalInput")
    o_d = nc.dram_tensor("out", [NT, 128, R], f32, kind="ExternalOutput")

    with tile.TileContext(nc) as tc:
        with (
            tc.tile_pool(name="qk", bufs=1) as qkpool,
            tc.tile_pool(name="small", bufs=1) as spool,
        ):
            eps_ln = spool.tile([128, 1], f32, tag="eps_ln")
            nc.vector.memset(eps_ln[:], LN_EPS)
            eps_mag = spool.tile([128, 1], f32, tag="eps_mag")
            nc.vector.memset(eps_mag[:], MAG_EPS2)
            p2a = spool.tile([128, 128], f16, tag="p2a")
            p2b = spool.tile([128, 128], f16, tag="p2b")
            imat = spool.tile([128, 128], f16, tag="imat")
            nc.sync.dma_start(p2a[:], p_d.ap()[0])
            nc.sync.dma_start(p2b[:], p_d.ap()[1])
            nc.sync.dma_start(imat[:], i_d.ap())

            # scale-1 megas, fp16 (filled per l-tile below)
            xn = {}
            for name in ("q", "k"):
                xn[(name, 0)] = qkpool.tile(
                    [128, NT, R], f16, tag=f"{name}mega", name=f"{name}mega"
                )
                for si, nkt in ((1, 6), (2, 3)):
                    xn[(name, si)] = qkpool.tile(
                        [128, nkt, R], f16, tag=f"{name}p{si}",
                        name=f"{name}p{si}",
                    )

            # ---- pipelined per-tile layernorm ----
            # q stats/normalize on DVE, k on GpSimd; squares+sqrt on ACT.
            with (
                tc.tile_pool(name="lnraw", bufs=4) as rpool,
                tc.tile_pool(name="lnstat", bufs=3) as stpool,
            ):
                for t in range(NT):
                    for name, src in (("q", q_d), ("k", k_d)):
                        raw = rpool.tile([128, R], f32, tag=f"raw{name}")
                        nc.sync.dma_start(raw[:], src.ap()[t])
                        x3 = raw[:].rearrange("p (h e) -> p h e", e=E)
                        sq = rpool.tile([128, R], f32, tag=f"sq{name}")
                        nc.scalar.activation(sq[:], raw[:], AF.Square)
                        st1 = stpool.tile([128, H], f32, tag=f"st1{name}")
                        nc.vector.tensor_reduce(
                            st1[:], x3, mybir.AxisListType.X, OP.add
                        )
                        st2 = stpool.tile([128, H], f32, tag=f"st2{name}")
                        nc.vector.tensor_reduce(
                            st2[:], sq[:].rearrange("p (h e) -> p h e", e=E),
                            mybir.AxisListType.X, OP.add,
                        )
                        mean = stpool.tile([128, H], f32, tag=f"mn{name}")
                        nc.vector.tensor_scalar_mul(mean[:], st1[:], 1.0 / E)
                        m2 = stpool.tile([128, H], f32, tag=f"m2{name}")
                        nc.vector.tensor_mul(m2[:], mean[:], mean[:])
                        var = stpool.tile([128, H], f32, tag=f"vr{name}")
                        nc.vector.scalar_tensor_tensor(
                            var[:], st2[:], 1.0 / E, m2[:],
                            op0=OP.mult, op1=OP.subtract,
                        )
                        std = stpool.tile([128, H], f32, tag=f"sd{name}")
                        nc.scalar.activation(
                            std[:], var[:], AF.Sqrt, bias=eps_ln[:]
                        )
                        rstd = stpool.tile([128, H], f32, tag=f"rs{name}")
                        nc.vector.reciprocal(rstd[:], std[:])
                        mr = stpool.tile([128, H], f32, tag=f"mr{name}")
                        nc.vector.tensor_mul(mr[:], mean[:], rstd[:])
                        # xn = x*rstd - mean*rstd  (2 ops on eng)
                        tmp = rpool.tile([128, R], f16, tag=f"tmp{name}")
                        t3 = tmp[:].rearrange("p (h e) -> p h e", e=E)
                        rstd3 = rstd[:].rearrange("p (h o) -> p h o", o=1)
                        x3b, rstd_b = bass.broadcast_tensor_aps(x3, rstd3)
                        nc.gpsimd.tensor_tensor(t3, x3b, rstd_b, OP.mult)
                        mg3 = xn[(name, 0)][:, t, :].rearrange(
                            "p (h e) -> p h e", e=E
                        )
                        mr3 = mr[:].rearrange("p (h o) -> p h o", o=1)
                        t3b, mr_b = bass.broadcast_tensor_aps(t3, mr3)
                        nc.gpsimd.tensor_tensor(mg3, t3b, mr_b, OP.subtract)

            # ---- DFT + spectral + lagged irfft (mean_corr) ----
            S16 = spool.tile([128, 32], f16, tag="s16")
            nc.vector.memset(S16[:], 0.0)
            with (
                tc.tile_pool(name="psum", bufs=5, space="PSUM") as pp,
                tc.tile_pool(name="mcpsum", bufs=1, space="PSUM") as mcp,
                tc.tile_pool(name="dstream", bufs=4) as dpool,
                tc.tile_pool(name="mstream", bufs=2) as mpool,
                tc.tile_pool(name="spec", bufs=2) as scp,
            ):
                mc_ps = [
                    mcp.tile([1, 512], f32, tag=f"mc{nt}", name=f"mc{nt}")
                    for nt in range(3)
                ]

                def emit_pools():
                    # avg-pool by 2 crosses partitions: PE packing matmuls
                    for name in ("q", "k"):
                        for si, nkt in ((1, 6), (2, 3)):
                            srcm = xn[(name, si - 1)]
                            dst = xn[(name, si)]
                            for j2 in range(nkt):
                                ps = pp.tile(
                                    [128, 512], f32, tag="dftps", name="poolps"
                                )
                                nc.tensor.matmul(
                                    ps[:], p2a[:], srcm[:, 2 * j2, :],
                                    start=True, stop=False,
                                )
                                nc.tensor.matmul(
                                    ps[:], p2b[:], srcm[:, 2 * j2 + 1, :],
                                    start=False, stop=True,
                                )
                                nc.scalar.activation(
                                    dst[:, j2, :], ps[:], AF.Copy
                                )

                pair_list = []
                for si in range(len(SCALES)):
                    reb, imb = _FT_BASE[si]
                    for j in range(FT[si]):
                        pair_list.append((si, j, reb + j, imb + j))
                n_pairs = len(pair_list)

                def is_orphan(si2, j2):
                    # last f-tile of scales 1,2 holds a single (Nyquist) bin
                    # whose imaginary part is exactly zero
                    return si2 < 2 and j2 == FT[si2] - 1

                def emit_mc(pi2, first_mm):
                    si2, j2, ftr2, fti2 = pair_list[pi2]
                    fts = (ftr2,) if is_orphan(si2, j2) else (ftr2, fti2)
                    for ft in fts:
                        mtile = mpool.tile([128, L], f16, tag="mtile")
                        nc.sync.dma_start(mtile[:], m_d.ap()[ft])
                        for nt in range(3):
                            nc.tensor.matmul(
                                mc_ps[nt][:], S16[:, ft : ft + 1],
                                mtile[:, nt * 512 : (nt + 1) * 512],
                                start=first_mm,
                                stop=(
                                    pi2 == n_pairs - 1 and ft == fts[-1]
                                    and nt == 2
                                ),
                                skip_group_check=True,
                            )
                        first_mm = False
                    return first_mm

                MC_LAG = 2
                first_mm = True
                for pi, (si, j, ftr, fti) in enumerate(pair_list):
                    nkt = KT[si]
                    qx = xn[("q", si)]
                    kx = xn[("k", si)]
                    orphan = is_orphan(si, j)
                    psl = {}
                    # load each D tile once; q chain then k chain share it
                    lf_list = (j,) if orphan else (j, FT[si] + j)
                    for li, lf in enumerate(lf_list):
                        part = "re" if li == 0 else "im"
                        dch = dpool.tile([128, nkt, 128], f16, tag=f"d{si}")
                        nc.sync.dma_start(
                            dch[:].rearrange("p a b -> p (a b)"),
                            d_ds[si].ap()[lf],
                        )
                        for nm, xm in ((f"q{part}", qx), (f"k{part}", kx)):
                            ps = pp.tile(
                                [128, 512], f32, tag="dftps", name=f"ps{nm}"
                            )
                            for kt in range(nkt):
                                nc.tensor.matmul(
                                    ps[:], dch[:, kt, :], xm[:, kt, :],
                                    start=(kt == 0), stop=(kt == nkt - 1),
                                )
                            sb = scp.tile([128, 512], f16, tag=f"{nm}S")
                            nc.scalar.activation(sb[:], ps[:], AF.Copy)
                            psl[nm] = sb
                    # mc matmuls, lagged so the PE never waits on spectral
                    if pi == 1:
                        emit_pools()
                    if pi >= MC_LAG:
                        first_mm = emit_mc(pi - MC_LAG, first_mm)
                    qreS, kreS = psl["qre"], psl["kre"]
                    sq1 = scp.tile([128, 512], f16, tag="sq1")
                    nc.scalar.activation(sq1[:], kreS[:], AF.Square)
                    if orphan:
                        # im == 0: S_re = sum qre*kre/|kre|, S_im = 0 (memset)
                        mag = scp.tile([128, 512], f32, tag="mag")
                        nc.scalar.activation(
                            mag[:], sq1[:], AF.Sqrt, bias=eps_mag[:, 0:1]
                        )
                        rs = scp.tile([128, 512], f32, tag="rs")
                        nc.vector.reciprocal_approx_fast(rs[:], mag[:])
                        khr = scp.tile([128, 512], f16, tag="khr")
                        nc.vector.tensor_mul(khr[:], kreS[:], rs[:])
                        scr = scp.tile([128, 512], f16, tag="scr")
                        a1 = scp.tile([128, 1], f32, tag="a1")
                        nc.vector.scalar_tensor_tensor(
                            scr[:], qreS[:], 0.0, khr[:], op0=OP.bypass,
                            op1=OP.mult, accum_out=a1[:],
                        )
                        nc.vector.tensor_copy(S16[:, ftr : ftr + 1], a1[:])
                        continue
                    qimS, kimS = psl["qim"], psl["kim"]
                    sq2 = scp.tile([128, 512], f16, tag="sq2")
                    nc.scalar.activation(sq2[:], kimS[:], AF.Square)
                    mag2 = scp.tile([128, 512], f16, tag="mag2")
                    nc.vector.tensor_add(mag2[:], sq1[:], sq2[:])
                    mag = scp.tile([128, 512], f32, tag="mag")
                    nc.scalar.activation(
                        mag[:], mag2[:], AF.Sqrt, bias=eps_mag[:, 0:1]
                    )
                    rs = scp.tile([128, 512], f32, tag="rs")
                    nc.vector.reciprocal_approx_fast(rs[:], mag[:])
                    khr = scp.tile([128, 512], f16, tag="khr")
                    khi = scp.tile([128, 512], f16, tag="khi")
                    nc.vector.tensor_mul(khr[:], kreS[:], rs[:])
                    nc.vector.tensor_mul(khi[:], kimS[:], rs[:])
                    scr = scp.tile([128, 512], f16, tag="scr")
                    scr2 = scp.tile([128, 512], f16, tag="scr2")
                    a1 = scp.tile([128, 1], f32, tag="a1")
                    a2 = scp.tile([128, 1], f32, tag="a2")
                    a3 = scp.tile([128, 1], f32, tag="a3")
                    a4 = scp.tile([128, 1], f32, tag="a4")
                    nc.vector.scalar_tensor_tensor(
                        scr[:], qreS[:], 0.0, khr[:], op0=OP.bypass,
                        op1=OP.mult, accum_out=a1[:],
                    )
                    nc.vector.scalar_tensor_tensor(
                        scr2[:], qimS[:], 0.0, khi[:], op0=OP.bypass,
                        op1=OP.mult, accum_out=a2[:],
                    )
                    nc.vector.tensor_add(S16[:, ftr : ftr + 1], a1[:], a2[:])
                    nc.vector.scalar_tensor_tensor(
                        scr[:], qimS[:], 0.0, khr[:], op0=OP.bypass,
                        op1=OP.mult, accum_out=a3[:],
                    )
                    nc.vector.scalar_tensor_tensor(
                        scr2[:], qreS[:], 0.0, khi[:], op0=OP.bypass,
                        op1=OP.mult, accum_out=a4[:],
                    )
                    nc.vector.tensor_sub(S16[:, fti : fti + 1], a3[:], a4[:])
                for pi in range(n_pairs - MC_LAG, n_pairs):
                    first_mm = emit_mc(pi, first_mm)

                mc_row = spool.tile([1, L], f32, tag="mcrow")
                for nt in range(3):
                    nc.vector.tensor_scalar_mul(
                        mc_row[:, nt * 512 : (nt + 1) * 512], mc_ps[nt][:],
                        MC_SHIFT,
                    )

            # ---- top-7 + softmax ----
            mc8 = spool.tile([1, 8], f32, tag="mc8")
            mcidx = spool.tile([1, 8], u32, tag="mcidx")
            nc.vector.max(mc8[:], mc_row[:])
            nc.vector.max_index(mcidx[:], mc8[:], mc_row[:])
            mc8c = spool.tile([1, 8], f32, tag="mc8c")
            nc.vector.tensor_copy(mc8c[:], mc8[:])
            mcidxc = spool.tile([1, 8], u32, tag="mcidxc")
            nc.vector.tensor_copy(mcidxc[:], mcidx[:])
            negmax = spool.tile([1, 1], f32, tag="negmax")
            nc.vector.tensor_scalar_mul(negmax[:], mc8c[:, 0:1], -1.0)
            e7 = spool.tile([1, TOPK], f32, tag="e7")
            nc.scalar.activation(e7[:], mc8c[:, 0:TOPK], AF.Exp, bias=negmax[:])
            ssum = spool.tile([1, 1], f32, tag="ssum")
            nc.vector.tensor_reduce(ssum[:], e7[:], mybir.AxisListType.X, OP.add)
            rsum = spool.tile([1, 1], f32, tag="rsum")
            nc.vector.reciprocal(rsum[:], ssum[:])
            nw = spool.tile([1, TOPK], f32, tag="nw")
            nc.vector.tensor_scalar_mul(nw[:], e7[:], rsum[:, 0:1])
            nw128 = spool.tile([128, TOPK], f32, tag="nw128")
            nc.gpsimd.partition_broadcast(nw128[:], nw[:])
            d128a = spool.tile([128, TOPK], u32, tag="d128a")
            nc.gpsimd.partition_broadcast(d128a[:], mcidxc[:, 0:TOPK])
            # weighted identity stationaries for the delay-MAC matmuls
            wI = []
            for kk in range(TOPK):
                wt = spool.tile([128, 128], f16, tag=f"wI{kk}", name=f"wI{kk}")
                nc.vector.tensor_scalar_mul(wt[:], imat[:], nw128[:, kk : kk + 1])
                wI.append(wt)
            iotas = []
            for g in range(NGRP):
                it = spool.tile([128, 1], u32, tag=f"iota{g}", name=f"iota{g}")
                nc.gpsimd.iota(
                    it[:], pattern=[[0, 1]], base=128 * GPK * g,
                    channel_multiplier=1,
                )
                iotas.append(it)

            # ---- gather (2 packed indirect gathers per delay) + PE MAC ----
            with (
                tc.tile_pool(name="gather", bufs=3) as gpool,
                tc.tile_pool(name="gpsum", bufs=6, space="PSUM") as gpp,
            ):
                acc = gpool.tile([128, NT, R], f32, tag="acc", bufs=1)
                for g in range(NGRP):
                    gps = [
                        gpp.tile([128, 512], f32, tag="gps", name=f"gps{g}_{c}")
                        for c in range(GPK)
                    ]
                    for kk in range(TOPK):
                        idx = gpool.tile([128, 1], u32, tag="idx")
                        nc.vector.tensor_tensor(
                            idx[:], iotas[g][:], d128a[:, kk : kk + 1], OP.add
                        )
                        slot = gpool.tile(
                            [128, GPK * R], f16, tag="slot", bufs=3
                        )
                        nc.gpsimd.indirect_dma_start(
                            out=slot[:],
                            out_offset=None,
                            in_=vw_d.ap(),
                            in_offset=bass.IndirectOffsetOnAxis(
                                ap=idx[:, 0:1], axis=0
                            ),
                        )
                        for c in range(GPK):
                            nc.tensor.matmul(
                                gps[c][:], wI[kk][:],
                                slot[:, c * R : (c + 1) * R],
                                start=(kk == 0), stop=(kk == TOPK - 1),
                            )
                    # drain + stream this group's output while next gathers
                    for c in range(GPK):
                        kt = GPK * g + c
                        nc.scalar.activation(acc[:, kt, :], gps[c][:], AF.Copy)
                        nc.sync.dma_start(o_d.ap()[kt], acc[:, kt, :])

    nc.compile()
    return nc


def _get_graph():
    if "nc" not in _CACHE:
        _CACHE["nc"] = _build_graph()
    return _CACHE["nc"]


def _make_in_maps(queries, keys, values, scale_weights, frequency_filter):
    d_chains, M_t, P2, I128 = _build_constants(
        np.asarray(scale_weights, np.float64),
        np.asarray(frequency_filter, np.float64),
    )
    q = np.ascontiguousarray(np.asarray(queries, np.float32).reshape(B, NT, 128, R))
    k = np.ascontiguousarray(np.asarray(keys, np.float32).reshape(B, NT, 128, R))
    v = np.asarray(values, np.float32).reshape(B, L, R)
    vv = np.concatenate([v, v], axis=1).astype(np.float16)  # [B, 2L, R]
    # sliding-window buffer: vw[b, i, c, :] = vv[b, i + 128*c, :], c < GPK
    st = vv.strides
    vw = np.lib.stride_tricks.as_strided(
        vv, shape=(B, NW, GPK, R), strides=(st[0], st[1], 128 * st[1], st[2])
    )
    in_maps = []
    for b in range(B):
        m = {
            "q": q[b],
            "k": k[b],
            "vw": np.ascontiguousarray(vw[b]).reshape(NW, GPK * R),
            "mmat": M_t,
        }
        m["pmat"] = P2
        m["imat"] = I128
        for si in range(len(SCALES)):
            m[f"dmat{si}"] = d_chains[si]
        in_maps.append(m)
    return in_maps


def kernel(queries, keys, values, scale_weights, frequency_filter, attn_mask=None):
    from concourse.bass_utils import run_bass_kernel_spmd

    nc = _get_graph()
    in_maps = _make_in_maps(queries, keys, values, scale_weights, frequency_filter)
    res = run_bass_kernel_spmd(nc, in_maps, core_ids=list(range(B)))
    out = np.stack(
        [np.asarray(res.results[b]["out"]).reshape(L, H, E) for b in range(B)]
    )
    return out.astype(np.float32)


# revision 6
# speedup vs baseline: 1.1595x; 1.0966x over previous
"""Trainium2 Bass kernel for nn_AdaptiveAutoCorrelation (8-core data-parallel).

v3 — engine-overhead-aware restructure (from 277us v2 / 293us v1):
  * LN in 3 chunks of 4 l-tiles: ACT casts raw f32 -> fp16 (+squares),
    DVE reduces at 2x fp16 rate with 3D APs, 5 small stat ops per
    (tensor, chunk), normalize via TT + one fused stt.  q/k interleaved,
    k-normalize on GpSimd.  First DFT matmul ~15us in (was 83us).
  * Avg-pool matmuls emitted before the DFT pair loop: the PE does them
    inside the LN-wait window.
  * Pair order puts a full scale-1 pair last so the lagged mean_corr
    matmuls and final spectral chain hide under its 10us of PE work.
  * D-chain tiles loaded once, shared by q and k chains.
  * Spectral whitening fp16 on DVE (f32 accum); psum drained by ACT.
  * Delay aggregation on PE (nw_k*I stationaries, PSUM accumulate);
    gather indices precomputed in one DVE op.
"""
import math

import numpy as np

L = 1536
H, E = 8, 64
R = H * E  # 512
B = 8
NT = L // 128  # 12 l-tiles
SCALES = [1, 2, 4]
KT = [12, 6, 3]  # contraction tiles per scale (pooled-first)
FBINS = [L // s // 2 + 1 for s in SCALES]  # [769, 385, 193]
FT = [(f + 127) // 128 for f in FBINS]  # f-tiles per re/im block: [7, 4, 2]
NFT = 2 * sum(FT)  # 26 total f-tiles
TOPK = int(math.log(L))  # 7
LN_EPS = 1e-5
GPK = 6  # tiles packed per gather row (6KB rows)
NGRP = NT // GPK  # 2 gathers per delay
NW = 2 * L - 128 * (GPK - 1)  # 2688 rows in the sliding-window gather buffer
MC_SHIFT = 2.0 ** -14  # undo host-side M prescale (exact power of two)
MAG_EPS2 = 1e-6  # |kf|^2 floor: keeps rs finite for zero-padded bins
LN_CHUNKS = [(0, 4), (4, 8), (8, 12)]

# global ftile index bases (for S / M layout): per scale, re tiles then im
_FT_BASE = []
_acc = 0
for _s in range(len(SCALES)):
    _FT_BASE.append((_acc, _acc + FT[_s]))
    _acc += 2 * FT[_s]

_CACHE = {}


def _build_constants(scale_weights, frequency_filter):
    """D chains per scale [2*nf,128,nkt*128] fp16, M [NFT,128,L] fp16."""
    f_sig = 1.0 / (1.0 + np.exp(-np.float64(frequency_filter[0])))
    sw = np.asarray(scale_weights[: len(SCALES)], np.float64)
    w = np.exp(sw - sw.max())
    w = w / w.sum()

    d_chains = []
    M = np.zeros((NFT * 128, L), np.float64)
    for si, s in enumerate(SCALES):
        Ls = L // s
        F = FBINS[si]
        nf = FT[si]
        nkt = KT[si]
        t = np.arange(Ls)[:, None]
        f = np.arange(F)[None, :]
        ang = 2.0 * np.pi * t * f / Ls
        Dre = np.zeros((Ls, nf * 128))
        Dim = np.zeros((Ls, nf * 128))
        Dre[:, :F] = np.cos(ang)
        Dim[:, :F] = -np.sin(ang)
        # chain layout: [lf, p, kt*128 + fc] = blk[kt*128 + p, j*128 + fc]
        cr = Dre.reshape(nkt, 128, nf, 128).transpose(2, 1, 0, 3)
        ci = Dim.reshape(nkt, 128, nf, 128).transpose(2, 1, 0, 3)
        ch = np.concatenate([cr, ci], axis=0).reshape(2 * nf, 128, nkt * 128)
        d_chains.append(np.ascontiguousarray(ch.astype(np.float16)))

        reb, imb = _FT_BASE[si]
        tt = np.arange(Ls)[None, :]
        cf = np.where((f.T == 0) | (f.T == F - 1), 1.0, 2.0)
        ang2 = 2.0 * np.pi * f.T * tt / Ls
        Mre = cf * np.cos(ang2) / Ls  # [F, Ls]
        Mim = -cf * np.sin(ang2) / Ls
        if Ls != L:
            P = np.zeros((Ls, L))
            co = np.clip((np.arange(L) + 0.5) * (Ls / L) - 0.5, 0, Ls - 1)
            lo = np.floor(co).astype(int)
            hi = np.minimum(lo + 1, Ls - 1)
            fr = co - lo
            P[lo, np.arange(L)] += 1 - fr
            P[hi, np.arange(L)] += fr
            Mre = Mre @ P
            Mim = Mim @ P
        scale = w[si] * f_sig / R * 16384.0  # 2^14 prescale for fp16 range
        M[reb * 128 : reb * 128 + F] = Mre * scale
        M[imb * 128 : imb * 128 + F] = Mim * scale

    M_t = np.ascontiguousarray(M.reshape(NFT, 128, L).astype(np.float16))
    # pool-by-2 packing matrices: P2a -> out cols [0,64), P2b -> [64,128)
    P2 = np.zeros((2, 128, 128), np.float16)
    for t_ in range(128):
        P2[0, t_, t_ // 2] = 0.5
        P2[1, t_, 64 + t_ // 2] = 0.5
    I128 = np.eye(128, dtype=np.float16)
    return d_chains, M_t, P2, I128


def _build_graph():
    import concourse.bacc as bacc
    import concourse.bass as bass
    import concourse.mybir as mybir
    import concourse.tile as tile

    AF = mybir.ActivationFunctionType
    OP = mybir.AluOpType
    f32 = mybir.dt.float32
    f16 = mybir.dt.float16
    u32 = mybir.dt.uint32

    nc = bacc.Bacc("TRN2", debug=False)
    q_d = nc.dram_tensor("q", [NT, 128, R], f32, kind="ExternalInput")
    k_d = nc.dram_tensor("k", [NT, 128, R], f32, kind="ExternalInput")
    vw_d = nc.dram_tensor("vw", [NW, GPK * R], f16, kind="ExternalInput")
    d_ds = [
        nc.dram_tensor(
            f"dmat{si}", [2 * FT[si], 128, KT[si] * 128], f16,
            kind="ExternalInput",
        )
        for si in range(len(SCALES))
    ]
    m_d = nc.dram_tensor("mmat", [NFT, 128, L], f16, kind="ExternalInput")
    p_d = nc.dram_tensor("pmat", [2, 128, 128], f16, kind="ExternalInput")
    i_d = nc.dram_tensor("imat", [128, 128], f16, kind="ExternalInput")
    o_d = nc.dram_tensor("out", [NT, 128, R], f32, kind="ExternalOutput")

    with tile.TileContext(nc) as tc:
        with (
            tc.tile_pool(name="qk", bufs=1) as qkpool,
            tc.tile_pool(name="small", bufs=1) as spool,
        ):
            eps_ln = spool.tile([128, 1], f32, tag="eps_ln")
            nc.vector.memset(eps_ln[:], LN_EPS)
            eps_mag = spool.tile([128, 1], f32, tag="eps_mag")
            nc.vector.memset(eps_mag[:], MAG_EPS2)
            p2a = spool.tile([128, 128], f16, tag="p2a")
            p2b = spool.tile([128, 128], f16, tag="p2b")
            imat = spool.tile([128, 128], f16, tag="imat")
            nc.sync.dma_start(p2a[:], p_d.ap()[0])
            nc.sync.dma_start(p2b[:], p_d.ap()[1])
            nc.sync.dma_start(imat[:], i_d.ap())
            iotas = []
            for g in range(NGRP):
                it = spool.tile([128, 1], u32, tag=f"iota{g}", name=f"iota{g}")
                nc.gpsimd.iota(
                    it[:], pattern=[[0, 1]], base=128 * GPK * g,
                    channel_multiplier=1,
                )
                iotas.append(it)

            xn = {}
            for name in ("q", "k"):
                xn[(name, 0)] = qkpool.tile(
                    [128, NT, R], f16, tag=f"{name}mega", name=f"{name}mega"
                )
                for si, nkt in ((1, 6), (2, 3)):
                    xn[(name, si)] = qkpool.tile(
                        [128, nkt, R], f16, tag=f"{name}p{si}",
                        name=f"{name}p{si}",
                    )

            # ---- chunked pipelined layernorm ----
            with (
                tc.tile_pool(name="lnraw", bufs=2) as rpool,
                tc.tile_pool(name="lnstat", bufs=2) as stpool,
            ):
                for c0, c1 in LN_CHUNKS:
                    ntc = c1 - c0
                    A = ntc * H
                    for name, src in (("q", q_d), ("k", k_d)):
                        raw = rpool.tile([128, ntc, R], f32, tag=f"raw{name}")
                        nc.sync.dma_start(
                            raw[:], src.ap()[c0:c1].rearrange("t p r -> p t r")
                        )
                        x16 = rpool.tile([128, ntc, R], f16, tag=f"x16{name}")
                        nc.scalar.activation(x16[:], raw[:], AF.Copy)
                        sq = rpool.tile([128, ntc, R], f16, tag=f"sq{name}")
                        nc.scalar.activation(sq[:], raw[:], AF.Square)
                        x3 = x16[:].rearrange("p t (h e) -> p (t h) e", e=E)
                        st1 = stpool.tile([128, A], f32, tag=f"st1{name}")
                        nc.vector.tensor_reduce(
                            st1[:], x3, mybir.AxisListType.X, OP.add
                        )
                        st2 = stpool.tile([128, A], f32, tag=f"st2{name}")
                        nc.vector.tensor_reduce(
                            st2[:],
                            sq[:].rearrange("p t (h e) -> p (t h) e", e=E),
                            mybir.AxisListType.X, OP.add,
                        )
                        mean = stpool.tile([128, A], f32, tag=f"mn{name}")
                        nc.vector.tensor_scalar_mul(mean[:], st1[:], 1.0 / E)
                        m2 = stpool.tile([128, A], f32, tag=f"m2{name}")
                        nc.vector.tensor_mul(m2[:], mean[:], mean[:])
                        var = stpool.tile([128, A], f32, tag=f"vr{name}")
                        nc.vector.scalar_tensor_tensor(
                            var[:], st2[:], 1.0 / E, m2[:],
                            op0=OP.mult, op1=OP.subtract,
                        )
                        std = stpool.tile([128, A], f32, tag=f"sd{name}")
                        nc.scalar.activation(
                            std[:], var[:], AF.Sqrt, bias=eps_ln[:]
                        )
                        rstd = stpool.tile([128, A], f32, tag=f"rs{name}")
                        nc.vector.reciprocal(rstd[:], std[:])
                        sr = stpool.tile([128, A], f32, tag=f"sr{name}")
                        nc.vector.tensor_mul(sr[:], mean[:], rstd[:])
                        # xhat = x16*rstd - sr  (TT + fused stt)
                        eng = nc.vector if name == "q" else nc.gpsimd
                        t1 = rpool.tile([128, ntc, R], f16, tag=f"t1{name}")
                        t13 = t1[:].rearrange("p t (h e) -> p (t h) e", e=E)
                        rstd3 = rstd[:].rearrange("p (a o) -> p a o", o=1)
                        x3b, rstd_b = bass.broadcast_tensor_aps(x3, rstd3)
                        eng.tensor_tensor(t13, x3b, rstd_b, OP.mult)
                        mg3 = xn[(name, 0)][:, c0:c1, :].rearrange(
                            "p t (h e) -> p (t h) e", e=E
                        )
                        sr3 = sr[:].rearrange("p (a o) -> p a o", o=1)
                        t13b, sr_b = bass.broadcast_tensor_aps(t13, sr3)
                        eng.tensor_tensor(mg3, t13b, sr_b, OP.subtract)

            # ---- DFT + spectral + lagged irfft (mean_corr) ----
            S16 = spool.tile([128, 32], f16, tag="s16")
            nc.vector.memset(S16[:], 0.0)
            with (
                tc.tile_pool(name="psum", bufs=5, space="PSUM") as pp,
                tc.tile_pool(name="mcpsum", bufs=1, space="PSUM") as mcp,
                tc.tile_pool(name="dstream", bufs=4) as dpool,
                tc.tile_pool(name="mstream", bufs=2) as mpool,
                tc.tile_pool(name="spec", bufs=2) as scp,
            ):
                mc_ps = [
                    mcp.tile([1, 512], f32, tag=f"mc{nt}", name=f"mc{nt}")
                    for nt in range(3)
                ]

                # pools first: the PE runs them inside the LN-wait window
                for name in ("q", "k"):
                    for si, nkt in ((1, 6), (2, 3)):
                        srcm = xn[(name, si - 1)]
                        dst = xn[(name, si)]
                        for j2 in range(nkt):
                            ps = pp.tile(
                                [128, 512], f32, tag="dftps", name="poolps"
                            )
                            nc.tensor.matmul(
                                ps[:], p2a[:], srcm[:, 2 * j2, :],
                                start=True, stop=False,
                            )
                            nc.tensor.matmul(
                                ps[:], p2b[:], srcm[:, 2 * j2 + 1, :],
                                start=False, stop=True,
                            )
                            nc.scalar.activation(dst[:, j2, :], ps[:], AF.Copy)

                def is_orphan(si2, j2):
                    # last f-tile of scales 1,2 holds a single (Nyquist) bin
                    # whose imaginary part is exactly zero
                    return si2 < 2 and j2 == FT[si2] - 1

                # big scale-1 pair last: its PE chains hide the mc tail
                pair_order = (
                    [(0, j) for j in (0, 1, 2, 3, 4, 6)]
                    + [(1, j) for j in range(FT[1])]
                    + [(2, j) for j in range(FT[2])]
                    + [(0, 5)]
                )
                pair_list = []
                for si, j in pair_order:
                    reb, imb = _FT_BASE[si]
                    pair_list.append((si, j, reb + j, imb + j))
                n_pairs = len(pair_list)

                def emit_mc(pi2, first_mm):
                    si2, j2, ftr2, fti2 = pair_list[pi2]
                    fts = (ftr2,) if is_orphan(si2, j2) else (ftr2, fti2)
                    for ft in fts:
                        mtile = mpool.tile([128, L], f16, tag="mtile")
                        nc.sync.dma_start(mtile[:], m_d.ap()[ft])
                        for nt in range(3):
                            nc.tensor.matmul(
                                mc_ps[nt][:], S16[:, ft : ft + 1],
                                mtile[:, nt * 512 : (nt + 1) * 512],
                                start=first_mm,
                                stop=(
                                    pi2 == n_pairs - 1 and ft == fts[-1]
                                    and nt == 2
                                ),
                                skip_group_check=True,
                            )
                        first_mm = False
                    return first_mm

                MC_LAG = 2
                first_mm = True
                for pi, (si, j, ftr, fti) in enumerate(pair_list):
                    nkt = KT[si]
                    qx = xn[("q", si)]
                    kx = xn[("k", si)]
                    orphan = is_orphan(si, j)
                    psl = {}
                    # load each D tile once; q chain then k chain share it
                    lf_list = (j,) if orphan else (j, FT[si] + j)
                    for li, lf in enumerate(lf_list):
                        part = "re" if li == 0 else "im"
                        dch = dpool.tile([128, nkt, 128], f16, tag=f"d{si}")
                        nc.sync.dma_start(
                            dch[:].rearrange("p a b -> p (a b)"),
                            d_ds[si].ap()[lf],
                        )
                        for nm, xm in ((f"q{part}", qx), (f"k{part}", kx)):
                            ps = pp.tile(
                                [128, 512], f32, tag="dftps", name=f"ps{nm}"
                            )
                            for kt in range(nkt):
                                nc.tensor.matmul(
                                    ps[:], dch[:, kt, :], xm[:, kt, :],
                                    start=(kt == 0), stop=(kt == nkt - 1),
                                )
                            sb = scp.tile([128, 512], f16, tag=f"{nm}S")
                            nc.scalar.activation(sb[:], ps[:], AF.Copy)
                            psl[nm] = sb
                    # mc matmuls, lagged so the PE never waits on spectral
                    if pi >= MC_LAG:
                        first_mm = emit_mc(pi - MC_LAG, first_mm)
                    qreS, kreS = psl["qre"], psl["kre"]
                    sq1 = scp.tile([128, 512], f16, tag="sq1")
                    nc.scalar.activation(sq1[:], kreS[:], AF.Square)
                    if orphan:
                        # im == 0: S_re = sum qre*kre/|kre|, S_im = 0 (memset)
                        mag = scp.tile([128, 512], f32, tag="mag")
                        nc.scalar.activation(
                            mag[:], sq1[:], AF.Sqrt, bias=eps_mag[:, 0:1]
                        )
                        rs = scp.tile([128, 512], f32, tag="rs")
                        nc.vector.reciprocal_approx_fast(rs[:], mag[:])
                        khr = scp.tile([128, 512], f16, tag="khr")
                        nc.vector.tensor_mul(khr[:], kreS[:], rs[:])
                        scr = scp.tile([128, 512], f16, tag="scr")
                        a1 = scp.tile([128, 1], f32, tag="a1")
                        nc.vector.scalar_tensor_tensor(
                            scr[:], qreS[:], 0.0, khr[:], op0=OP.bypass,
                            op1=OP.mult, accum_out=a1[:],
                        )
                        nc.vector.tensor_copy(S16[:, ftr : ftr + 1], a1[:])
                        continue
                    qimS, kimS = psl["qim"], psl["kim"]
                    sq2 = scp.tile([128, 512], f16, tag="sq2")
                    nc.vector.scalar_tensor_tensor(
                        sq2[:], kimS[:], 0.0, kimS[:], op0=OP.bypass,
                        op1=OP.mult,
                    )
                    mag2 = scp.tile([128, 512], f16, tag="mag2")
                    nc.vector.tensor_add(mag2[:], sq1[:], sq2[:])
                    mag = scp.tile([128, 512], f32, tag="mag")
                    nc.scalar.activation(
                        mag[:], mag2[:], AF.Sqrt, bias=eps_mag[:, 0:1]
                    )
                    rs = scp.tile([128, 512], f32, tag="rs")
                    nc.vector.reciprocal_approx_fast(rs[:], mag[:])
                    khr = scp.tile([128, 512], f16, tag="khr")
                    khi = scp.tile([128, 512], f16, tag="khi")
                    nc.vector.tensor_mul(khr[:], kreS[:], rs[:])
                    nc.vector.tensor_mul(khi[:], kimS[:], rs[:])
                    scr = scp.tile([128, 512], f16, tag="scr")
                    scr2 = scp.tile([128, 512], f16, tag="scr2")
                    a1 = scp.tile([128, 1], f32, tag="a1")
                    a2 = scp.tile([128, 1], f32, tag="a2")
                    a3 = scp.tile([128, 1], f32, tag="a3")
                    a4 = scp.tile([128, 1], f32, tag="a4")
                    nc.vector.scalar_tensor_tensor(
                        scr[:], qreS[:], 0.0, khr[:], op0=OP.bypass,
                        op1=OP.mult, accum_out=a1[:],
                    )
                    nc.vector.scalar_tensor_tensor(
                        scr2[:], qimS[:], 0.0, khi[:], op0=OP.bypass,
                        op1=OP.mult, accum_out=a2[:],
                    )
                    nc.vector.tensor_add(S16[:, ftr : ftr + 1], a1[:], a2[:])
                    nc.vector.scalar_tensor_tensor(
                        scr[:], qimS[:], 0.0, khr[:], op0=OP.bypass,
                        op1=OP.mult, accum_out=a3[:],
                    )
                    nc.vector.scalar_tensor_tensor(
                        scr2[:], qreS[:], 0.0, khi[:], op0=OP.bypass,
                        op1=OP.mult, accum_out=a4[:],
                    )
                    nc.vector.tensor_sub(S16[:, fti : fti + 1], a3[:], a4[:])
                for pi in range(n_pairs - MC_LAG, n_pairs):
                    first_mm = emit_mc(pi, first_mm)

                mc_row = spool.tile([1, L], f32, tag="mcrow")
                for nt in range(3):
                    nc.vector.tensor_scalar_mul(
                        mc_row[:, nt * 512 : (nt + 1) * 512], mc_ps[nt][:],
                        MC_SHIFT,
                    )

            # ---- top-7 + softmax ----
            mc8 = spool.tile([1, 8], f32, tag="mc8")
            mcidx = spool.tile([1, 8], u32, tag="mcidx")
            nc.vector.max(mc8[:], mc_row[:])
            nc.vector.max_index(mcidx[:], mc8[:], mc_row[:])
            mc8c = spool.tile([1, 8], f32, tag="mc8c")
            nc.vector.tensor_copy(mc8c[:], mc8[:])
            mcidxc = spool.tile([1, 8], u32, tag="mcidxc")
            nc.vector.tensor_copy(mcidxc[:], mcidx[:])
            negmax = spool.tile([1, 1], f32, tag="negmax")
            nc.vector.tensor_scalar_mul(negmax[:], mc8c[:, 0:1], -1.0)
            e7 = spool.tile([1, TOPK], f32, tag="e7")
            nc.scalar.activation(e7[:], mc8c[:, 0:TOPK], AF.Exp, bias=negmax[:])
            ssum = spool.tile([1, 1], f32, tag="ssum")
            nc.vector.tensor_reduce(ssum[:], e7[:], mybir.AxisListType.X, OP.add)
            rsum = spool.tile([1, 1], f32, tag="rsum")
            nc.vector.reciprocal(rsum[:], ssum[:])
            nw = spool.tile([1, TOPK], f32, tag="nw")
            nc.vector.tensor_scalar_mul(nw[:], e7[:], rsum[:, 0:1])
            nw128 = spool.tile([128, TOPK], f32, tag="nw128")
            nc.gpsimd.partition_broadcast(nw128[:], nw[:])
            d128a = spool.tile([128, TOPK], u32, tag="d128a")
            nc.gpsimd.partition_broadcast(d128a[:], mcidxc[:, 0:TOPK])
            # all 2*7 gather indices in one DVE op
            iota2 = spool.tile([128, NGRP], u32, tag="iota2")
            for g in range(NGRP):
                nc.vector.tensor_copy(iota2[:, g : g + 1], iotas[g][:])
            idx_all = spool.tile([128, NGRP, TOPK], u32, tag="idxall")
            ii = iota2[:].rearrange("p (g o) -> p g o", o=1)
            dd = d128a[:].rearrange("p (o k) -> p o k", o=1)
            import concourse.bass as bass2

            iib, ddb = bass2.broadcast_tensor_aps(ii, dd)
            nc.vector.tensor_tensor(idx_all[:], iib, ddb, OP.add)
            # weighted identity stationaries for the delay-MAC matmuls
            wI = []
            for kk in range(TOPK):
                wt = spool.tile([128, 128], f16, tag=f"wI{kk}", name=f"wI{kk}")
                nc.vector.tensor_scalar_mul(wt[:], imat[:], nw128[:, kk : kk + 1])
                wI.append(wt)

            # ---- gather (2 packed indirect gathers per delay) + PE MAC ----
            with (
                tc.tile_pool(name="gather", bufs=3) as gpool,
                tc.tile_pool(name="gpsum", bufs=6, space="PSUM") as gpp,
            ):
                acc = gpool.tile([128, NT, R], f32, tag="acc", bufs=1)
                for g in range(NGRP):
                    gps = [
                        gpp.tile([128, 512], f32, tag="gps", name=f"gps{g}_{c}")
                        for c in range(GPK)
                    ]
                    for kk in range(TOPK):
                        slot = gpool.tile(
                            [128, GPK * R], f16, tag="slot", bufs=4
                        )
                        nc.gpsimd.indirect_dma_start(
                            out=slot[:],
                            out_offset=None,
                            in_=vw_d.ap(),
                            in_offset=bass.IndirectOffsetOnAxis(
                                ap=idx_all[:, g, kk : kk + 1], axis=0
                            ),
                        )
                        for c in range(GPK):
                            nc.tensor.matmul(
                                gps[c][:], wI[kk][:],
                                slot[:, c * R : (c + 1) * R],
                                start=(kk == 0), stop=(kk == TOPK - 1),
                            )
                    # drain + stream this group's output while next gathers
                    for c in range(GPK):
                        kt = GPK * g + c
                        nc.scalar.activation(acc[:, kt, :], gps[c][:], AF.Copy)
                        nc.sync.dma_start(o_d.ap()[kt], acc[:, kt, :])

    nc.compile()
    return nc


def _get_graph():
    if "nc" not in _CACHE:
        _CACHE["nc"] = _build_graph()
    return _CACHE["nc"]


def _make_in_maps(queries, keys, values, scale_weights, frequency_filter):
    d_chains, M_t, P2, I128 = _build_constants(
        np.asarray(scale_weights, np.float64),
        np.asarray(frequency_filter, np.float64),
    )
    q = np.ascontiguousarray(np.asarray(queries, np.float32).reshape(B, NT, 128, R))
    k = np.ascontiguousarray(np.asarray(keys, np.float32).reshape(B, NT, 128, R))
    v = np.asarray(values, np.float32).reshape(B, L, R)
    vv = np.concatenate([v, v], axis=1).astype(np.float16)  # [B, 2L, R]
    # sliding-window buffer: vw[b, i, c, :] = vv[b, i + 128*c, :], c < GPK
    st = vv.strides
    vw = np.lib.stride_tricks.as_strided(
        vv, shape=(B, NW, GPK, R), strides=(st[0], st[1], 128 * st[1], st[2])
    )
    in_maps = []
    for b in range(B):
        m = {
            "q": q[b],
            "k": k[b],
            "vw": np.ascontiguousarray(vw[b]).reshape(NW, GPK * R),
            "mmat": M_t,
        }
        m["pmat"] = P2
        m["imat"] = I128
        for si in range(len(SCALES)):
            m[f"dmat{si}"] = d_chains[si]
        in_maps.append(m)
    return in_maps


def kernel(queries, keys, values, scale_weights, frequency_filter, attn_mask=None):
    from concourse.bass_utils import run_bass_kernel_spmd

    nc = _get_graph()
    in_maps = _make_in_maps(queries, keys, values, scale_weights, frequency_filter)
    res = run_bass_kernel_spmd(nc, in_maps, core_ids=list(range(B)))
    out = np.stack(
        [np.asarray(res.results[b]["out"]).reshape(L, H, E) for b in range(B)]
    )
    return out.astype(np.float32)
